# revision 3
# baseline (speedup 1.0000x reference)
"""Trainium2 kernel for nn_DictionaryLearning (FISTA / LISTA-style loop).

Reference computation per data column n (independent across n — pure data
parallel over the 32768 columns, sharded 8 ways):

    product P_m = operator_m @ D                  (4, 64, 128)
    G_m = P_m^T P_m ; lip = max_m ||G_m||_F ; step = 1/lip ; thr = step*lambd
    z_k = (I - step*G) @ out_k + step*P^T y       (momentum state out_k)
    it_{k+1} = softshrink(z_k, thr)
    out_{k+1} = (1+mu_{k+1}) it_{k+1} - mu_{k+1} it_k

Device mapping (validated against reference to ~1.6e-5 in fp64/numpy proto):
  - host precomputes  b = step*P^T y  (uploaded per-core, sharded on columns)
    and per-iteration weight pairs  W1_k = (1+mu_k)*A, W2_k = -mu_k*A  with
    A = I - step*G  (momentum folded into the PE weights; A symmetric).
  - per iteration: PSUM z' = W1_k @ it_k + W2_k @ it_{k-1}   (float32r matmuls)
    then ONE fused custom-DVE op: it_{k+1} = softshrink(z' + b, thr).
  - final: out = (1+mu_f) it_30 - mu_f it_29  (fused MOMBINE op), DMA out.

Engine budget per core-iteration (E = 16384 elem/lane): PE 2 passes
(~13.7us), DVE 1 fused pass (~17.3us) -> DVE-bound ~520us total.
"""

import sys

if "/opt/trn_rl_repo" not in sys.path:
    sys.path.insert(0, "/opt/trn_rl_repo")

import numpy as np

import concourse.bacc as bacc
import concourse.mybir as mybir
import concourse.tile as tile
from concourse import bass_utils
from concourse.dve_ops import (
    OPS,
    CUSTOM_DVE_SPECS,
    _SUB_OPCODE_FOR_NAME,
    DveOp,
    has_src1,
)
from concourse.dve_spec import Spec, Src0, Src1, C0, C1, C2, maxx, minn, lower
from concourse.dve_uop import DveOpSpec

LAMBD = 0.1
N_CORES = 8
M_MAT, DY, DX = 4, 64, 128
N_DATA = 32768
NSH = N_DATA // N_CORES        # 4096 columns per core
CHUNK = 2048                   # columns per PSUM tile / DVE op
SUB = 512                      # columns per matmul (one PSUM bank, fp32)
F32 = mybir.dt.float32
F32R = mybir.dt.float32r


def _register(name, spec, subdim=False):
    """Register a custom DVE op at import time with self-pinned uop shas."""
    if name in _SUB_OPCODE_FOR_NAME:
        return next(op for op in OPS if op.name == name)
    shas = {}
    for ver in ("v3", "v4"):
        s = DveOpSpec(name=name, opcode=0, uops=lower(spec, ver=ver),
                      rd1_en=has_src1(spec))
        shas[ver] = s.sha(ver)
    op = DveOp(name, spec, subdim=subdim, uops_sha=shas)
    OPS.append(op)
    _SUB_OPCODE_FOR_NAME[name] = max(_SUB_OPCODE_FOR_NAME.values()) + 1
    assert _SUB_OPCODE_FOR_NAME[name] < 0x20
    CUSTOM_DVE_SPECS[name] = spec
    return op


# it = softshrink(in0 + s0*in1, thr) : s1 = +thr, imm2 = -thr
SHRINK_AFF = _register(
    "SHRINK_AFF",
    Spec(
        body=(lambda z: z - maxx(minn(z, C1), C2))(Src0 + C0 * Src1),
        reference=lambda in0, in1, s0, s1, imm2: (
            lambda z: (z - np.maximum(np.minimum(z, s1), imm2)).astype(np.float32)
        )(in0 + s0 * in1),
    ),
)

# out = s0*in0 + s1*in1   (final momentum extrapolation)
MOMBINE = _register(
    "MOMBINE",
    Spec(
        body=C0 * Src0 + C1 * Src1,
        reference=lambda in0, in1, s0, s1, imm2: (s0 * in0 + s1 * in1).astype(
            np.float32
        ),
    ),
)


def _host_precompute(y, operator, D, max_iter):
    """Mirror the reference's fp32 scalar/matrix computations in numpy."""
    y = np.asarray(y, np.float32)
    operator = np.asarray(operator, np.float32)
    D = np.asarray(D, np.float32)

    prod = operator @ D                                   # (M, 64, 128)
    gram = np.einsum("mij,mik->mjk", prod, prod).astype(np.float32)
    lip = np.sqrt((gram ** 2).sum(axis=(1, 2))).max()
    step = np.float32(1.0) / np.float32(lip)
    thr = np.float32(step * np.float32(LAMBD))

    A = np.eye(DX, dtype=np.float32)[None] - step * gram  # (M, 128, 128)
    b = step * np.einsum("mix,min->mxn", prod, y)         # (M, 128, N)

    ts = [np.float32(1.0)]
    for _ in range(max_iter + 1):
        ts.append(np.float32(0.5 * (1.0 + np.sqrt(1.0 + 4.0 * ts[-1] ** 2))))
    mus = [np.float32(0.0)] + [
        np.float32((ts[k] - 1.0) / ts[k + 1]) for k in range(max_iter)
    ]

    # Weights, laid out DMA-friendly: wts[m, i(K-part), kk, j, c] with
    # lhsT[i, c] = W[c, i] (W symmetric, but store the transpose explicitly).
    n_w = max(max_iter - 1, 0)
    wts = np.zeros((M_MAT, DX, n_w, 2, DX), np.float32)
    for k in range(1, max_iter):
        W1 = ((1.0 + mus[k]) * A).astype(np.float32)
        W2 = (-mus[k] * A).astype(np.float32)
        for m in range(M_MAT):
            wts[m, :, k - 1, 0, :] = W1[m].T
            wts[m, :, k - 1, 1, :] = W2[m].T
    return b, wts, thr, mus


def _build_nc(max_iter, thr, mu_f, n_w, repeat=1):
    """Build the per-core bass module (SPMD: same program on all 8 cores)."""
    nc = bacc.Bacc(None, target_bir_lowering=False)
    b_d = nc.dram_tensor("b", (M_MAT, DX, NSH), F32, kind="ExternalInput")
    w_d = nc.dram_tensor("wts", (M_MAT, DX, n_w, 2, DX), F32R, kind="ExternalInput")
    o_d = nc.dram_tensor("out", (M_MAT, DX, NSH), F32, kind="ExternalOutput")

    n_chunk = NSH // CHUNK
    n_sub = CHUNK // SUB
    thr = float(thr)
    mu_f = float(mu_f)

    with tile.TileContext(nc) as tc:
        with (
            tc.tile_pool(name="it", bufs=3) as it_pool,
            tc.tile_pool(name="bb", bufs=2) as b_pool,
            tc.tile_pool(name="ww", bufs=2) as w_pool,
            tc.tile_pool(name="oo", bufs=2) as o_pool,
            tc.tile_pool(name="ps", bufs=2, space="PSUM") as ps_pool,
        ):
            for _ in range(repeat):
                for m in range(M_MAT):
                    b_t = b_pool.tile([DX, NSH], F32, tag="b", name=f"b{m}")
                    w_t = w_pool.tile([DX, n_w, 2, DX], F32R, tag="w", name=f"w{m}")
                    o_t = o_pool.tile([DX, NSH], F32, tag="o", name=f"o{m}")
                    nc.sync.dma_start(b_t[:], b_d[m])
                    nc.sync.dma_start(w_t[:], w_d[m])

                    its = [it_pool.tile([DX, NSH], F32R, tag="it", name=f"it{m}_{i}") for i in range(3)]

                    # k = 0: it_1 = shrink(b)
                    for c in range(n_chunk):
                        cs = slice(c * CHUNK, (c + 1) * CHUNK)
                        nc.vector._custom_dve(
                            SHRINK_AFF, out=its[1][:, cs], in0=b_t[:, cs],
                            in1=b_t[:, cs], s0=0.0, s1=thr, imm2=-thr,
                        )

                    # k = 1 .. max_iter-1
                    for k in range(1, max_iter):
                        cur = its[k % 3]
                        prev = its[(k - 1) % 3]
                        nxt = its[(k + 1) % 3]
                        for c in range(n_chunk):
                            pc = ps_pool.tile([DX, CHUNK], F32, tag="z", name=f"z{m}_{k}_{c}")
                            for s in range(n_sub):
                                col = c * CHUNK + s * SUB
                                ps_s = pc[:, s * SUB:(s + 1) * SUB]
                                rhs1 = cur[:, col:col + SUB]
                                if k == 1:
                                    nc.tensor.matmul(
                                        ps_s, w_t[:, k - 1, 0, :], rhs1,
                                        start=True, stop=True,
                                    )
                                else:
                                    rhs2 = prev[:, col:col + SUB]
                                    nc.tensor.matmul(
                                        ps_s, w_t[:, k - 1, 0, :], rhs1,
                                        start=True, stop=False,
                                    )
                                    nc.tensor.matmul(
                                        ps_s, w_t[:, k - 1, 1, :], rhs2,
                                        start=False, stop=True,
                                    )
                            cs = slice(c * CHUNK, (c + 1) * CHUNK)
                            nc.vector._custom_dve(
                                SHRINK_AFF, out=nxt[:, cs], in0=pc[:],
                                in1=b_t[:, cs], s0=1.0, s1=thr, imm2=-thr,
                            )

                    it_last = its[max_iter % 3]
                    it_prev = its[(max_iter - 1) % 3]
                    for c in range(n_chunk):
                        cs = slice(c * CHUNK, (c + 1) * CHUNK)
                        nc.vector._custom_dve(
                            MOMBINE, out=o_t[:, cs], in0=it_last[:, cs],
                            in1=it_prev[:, cs], s0=1.0 + mu_f, s1=-mu_f,
                        )
                    nc.sync.dma_start(o_d[m], o_t[:])
    nc.compile()
    return nc


_NC_CACHE = {}


def _get_nc(max_iter, thr, mu_f, n_w, repeat=1):
    key = (max_iter, float(thr), float(mu_f), n_w, repeat)
    if key not in _NC_CACHE:
        _NC_CACHE[key] = _build_nc(max_iter, thr, mu_f, n_w, repeat)
    return _NC_CACHE[key]


def kernel(y, operator, D, max_iter, _repeat=1):
    max_iter = int(max_iter)
    y = np.asarray(y, np.float32)
    assert y.shape == (M_MAT, DY, N_DATA) and max_iter >= 2

    b, wts, thr, mus = _host_precompute(y, operator, D, max_iter)
    mu_f = mus[max_iter]
    n_w = max_iter - 1

    nc = _get_nc(max_iter, thr, mu_f, n_w, _repeat)

    in_maps = []
    for c in range(N_CORES):
        sl = slice(c * NSH, (c + 1) * NSH)
        in_maps.append({
            "b": np.ascontiguousarray(b[:, :, sl]),
            "wts": wts,
        })
    res = bass_utils.run_bass_kernel_spmd(nc, in_maps, core_ids=list(range(N_CORES)))
    out = np.concatenate([res.results[c]["out"] for c in range(N_CORES)], axis=2)
    return out.astype(np.float32)
